# revision 18
# baseline (speedup 1.0000x reference)
"""Trainium2 Bass kernel for the FLUX-style joint-attention block.

Sharding: tensor-parallel over heads — 24 heads / 8 cores = 3 heads per core.
Each core computes QKV projections for its heads (fp16 matmuls, fp32 PSUM),
per-head RMSNorm + RoPE (g folded into host-prepped rope coefficients, head dim
de-interleaved so rope reads contiguous halves), full joint attention over the
2304-token sequence in a scores-transposed layout (softmax denominator via an
all-ones matmul, exp shifted by -6 so fp16 probabilities cannot overflow), and
its contraction-shard of the output projections. The host sums the 8 partial
outputs and adds biases.
"""
import numpy as np
from contextlib import ExitStack

import concourse.bacc as bacc
import concourse.mybir as mybir
from concourse.tile import TileContext
from concourse.masks import make_identity
from concourse.bass_utils import run_bass_kernel_spmd

B, S_IMG, S_TXT = 1, 2048, 256
S = S_IMG + S_TXT
DM, H, DH = 3072, 24, 128
EPS = 1e-5
NCORES, HL = 8, 3          # cores, heads per core
NT, NCH = S // 128, DM // 128   # 18 seq tiles, 24 dm chunks
NTT = S_TXT // 128         # 2 txt seq tiles (joint order: txt first)
QB, NQB = 384, S // 384    # attention q-block
OC, NOC = 512, DM // 512   # out-proj column chunk
EXP_SHIFT = -6.0
SCALE = 1.0 / np.sqrt(DH)

PERM = np.concatenate([np.arange(0, DH, 2), np.arange(1, DH, 2)])

_cached = {}


def _build_nc(phases=3, p1sub="full"):
    f16, f32 = mybir.dt.float16, mybir.dt.float32
    mult, add = mybir.AluOpType.mult, mybir.AluOpType.add
    Sqrt = mybir.ActivationFunctionType.Sqrt
    Exp = mybir.ActivationFunctionType.Exp

    nc = bacc.Bacc("TRN2", target_bir_lowering=False)
    x_d = nc.dram_tensor("x", [NT, 128, NCH, 128], f16, kind="ExternalInput")
    wi_d = nc.dram_tensor("wimg", [128, NCH, 3 * HL * DH], f16, kind="ExternalInput")
    wt_d = nc.dram_tensor("wtxt", [128, NCH, 3 * HL * DH], f16, kind="ExternalInput")
    rq_d = nc.dram_tensor("ropeq", [NT, 128, 4, 64], f16, kind="ExternalInput")
    rk_d = nc.dram_tensor("ropek", [NT, 128, 4, 64], f16, kind="ExternalInput")
    woi_d = nc.dram_tensor("woi", [128, HL, DM], f16, kind="ExternalInput")
    wot_d = nc.dram_tensor("wot", [128, HL, DM], f16, kind="ExternalInput")
    out_d = nc.dram_tensor("out", [S, DM], f32, kind="ExternalOutput")
    dbg_d = None
    if phases < 3:
        dbg_d = nc.dram_tensor("dbg", [4, 128, HL * S], f16, kind="ExternalOutput")

    with TileContext(nc) as tc, ExitStack() as ctx:
        const = ctx.enter_context(tc.tile_pool(name="const", bufs=1))
        ident = const.tile([128, 128], f16)
        make_identity(nc, ident)
        ones = const.tile([128, 128], f16)
        nc.vector.memset(ones, 1.0)
        eps_c = const.tile([128, 1], f32)
        nc.vector.memset(eps_c, EPS)
        shift_c = const.tile([128, 1], f32)
        nc.vector.memset(shift_c, EXP_SHIFT)

        big = ctx.enter_context(tc.tile_pool(name="big", bufs=1))
        qT = big.tile([128, HL, S], f16, tag="qT")
        kT = big.tile([128, HL, S], f16, tag="kT")
        vsb = big.tile([128, NT, HL, DH], f16, tag="vsb")

        # ---------------- phase 1: QKV + RMSNorm + RoPE + transpose ----------
        with (
            tc.tile_pool(name="wp", bufs=1) as wp,
            tc.tile_pool(name="xp", bufs=2) as xp,
            tc.tile_pool(name="rp", bufs=2) as rp,
            tc.tile_pool(name="tmp", bufs=3) as tmp,
            tc.tile_pool(name="qkvps", bufs=2, space="PSUM") as qkvps,
            tc.tile_pool(name="tps", bufs=2, space="PSUM") as tps,
        ):
            wgs = {}
            for nm, d in (("t", wt_d), ("i", wi_d)):
                for g in range(4):
                    w = wp.tile([128, 6, 3 * HL * DH], f16, tag=f"w{nm}{g}")
                    nc.sync.dma_start(out=w, in_=d[:, 6 * g:6 * (g + 1), :])
                    wgs[(nm, g)] = w

            for t in range(NT):
                stream = "t" if t < NTT else "i"
                xt = xp.tile([128, NCH, 128], f16, tag="x")
                nc.sync.dma_start(out=xt, in_=x_d[t])
                rqt = rp.tile([128, 4, 64], f16, tag="rq")
                nc.sync.dma_start(out=rqt, in_=rq_d[t])
                rkt = rp.tile([128, 4, 64], f16, tag="rk")
                nc.sync.dma_start(out=rkt, in_=rk_d[t])

                psq = qkvps.tile([128, HL * DH], f32, tag="q")
                psk = qkvps.tile([128, HL * DH], f32, tag="k")
                psv = qkvps.tile([128, HL * DH], f32, tag="v")
                n = HL * DH
                for c in range(NCH):
                    w = wgs[(stream, c // 6)]
                    lhs = xt[:, c, :]
                    st, sp = (c == 0), (c == NCH - 1)
                    nc.tensor.matmul(psq, lhsT=lhs, rhs=w[:, c % 6, 0:n], start=st, stop=sp)
                    nc.tensor.matmul(psk, lhsT=lhs, rhs=w[:, c % 6, n:2 * n], start=st, stop=sp)
                    nc.tensor.matmul(psv, lhsT=lhs, rhs=w[:, c % 6, 2 * n:3 * n], start=st, stop=sp)

                nc.scalar.copy(out=vsb[:, t], in_=psv.rearrange("p (h d) -> p h d", h=HL))

                if p1sub == "mm":
                    nc.scalar.copy(out=qT[:, :, t * 128:(t + 1) * 128],
                                   in_=psq.rearrange("p (h d) -> p h d", h=HL))
                    nc.scalar.copy(out=kT[:, :, t * 128:(t + 1) * 128],
                                   in_=psk.rearrange("p (h d) -> p h d", h=HL))
                    continue
                for h in range(HL):
                    for ps, rt, dstT in ((psq, rqt, qT), (psk, rkt, kT)):
                        xs = tmp.tile([128, DH], f32, tag="xs")
                        nc.scalar.copy(out=xs, in_=ps[:, h * DH:(h + 1) * DH])
                        sqd = tmp.tile([128, DH], f32, tag="sqd")
                        nc.vector.tensor_mul(sqd, xs, xs)
                        ssum = tmp.tile([128, 1], f32, tag="ssum")
                        nc.vector.reduce_sum(out=ssum, in_=sqd, axis=mybir.AxisListType.X)
                        srt = tmp.tile([128, 1], f32, tag="srt")
                        nc.scalar.activation(srt, ssum, Sqrt, bias=eps_c, scale=1.0 / DH)
                        rstd = tmp.tile([128, 1], f32, tag="rstd")
                        nc.vector.reciprocal(rstd, srt)

                        xn = tmp.tile([128, DH], f32, tag="xn")
                        nc.vector.tensor_scalar_mul(xn, xs, rstd)
                        rot = tmp.tile([128, DH], f16, tag="rot")
                        t1 = tmp.tile([128, 64], f32, tag="t1")
                        t2 = tmp.tile([128, 64], f32, tag="t2")
                        nc.vector.tensor_mul(t1, xn[:, 0:64], rt[:, 0])
                        nc.vector.tensor_mul(t2, xn[:, 64:128], rt[:, 1])
                        nc.vector.tensor_add(rot[:, 0:64], t1, t2)
                        t3 = tmp.tile([128, 64], f32, tag="t3")
                        t4 = tmp.tile([128, 64], f32, tag="t4")
                        nc.vector.tensor_mul(t3, xn[:, 0:64], rt[:, 2])
                        nc.vector.tensor_mul(t4, xn[:, 64:128], rt[:, 3])
                        nc.vector.tensor_add(rot[:, 64:128], t3, t4)

                        if p1sub == "norm":
                            nc.scalar.copy(out=dstT[:, h, t * 128:(t + 1) * 128], in_=rot)
                        else:
                            ptp = tps.tile([128, 128], f16, tag="tp")
                            nc.tensor.transpose(ptp, rot, ident)
                            nc.scalar.copy(out=dstT[:, h, t * 128:(t + 1) * 128], in_=ptp)

        if phases == 1:
            nc.sync.dma_start(out=dbg_d[0], in_=qT.rearrange("p h s -> p (h s)"))
            nc.sync.dma_start(out=dbg_d[1], in_=kT.rearrange("p h s -> p (h s)"))
            nc.sync.dma_start(out=dbg_d[2], in_=vsb.rearrange("p t h d -> p (t h d)"))

        # ---------------- phase 2: attention --------------------------------
        attnT = None
        if phases >= 2:
            attnT = big.tile([128, HL, S], f16, tag="attnT")
        with (
            tc.tile_pool(name="pp", bufs=3) as pp,
            tc.tile_pool(name="rcp", bufs=2) as rcp,
            tc.tile_pool(name="sps", bufs=3, space="PSUM") as sps,
            tc.tile_pool(name="aps", bufs=2, space="PSUM") as aps,
            tc.tile_pool(name="dps", bufs=2, space="PSUM") as dps,
        ):
            for h in range(HL if phases >= 2 else 0):
                for qb in range(NQB):
                    qsl = slice(qb * QB, (qb + 1) * QB)
                    att = aps.tile([128, QB], f32, tag="a")
                    den = dps.tile([128, QB], f32, tag="d")
                    for jt in range(NT):
                        sc = sps.tile([128, QB], f32, tag="s")
                        nc.tensor.matmul(
                            sc, lhsT=kT[:, h, jt * 128:(jt + 1) * 128],
                            rhs=qT[:, h, qsl], start=True, stop=True)
                        pt = pp.tile([128, QB], f16, tag="p")
                        nc.scalar.activation(pt, sc, Exp, bias=shift_c, scale=SCALE)
                        nc.tensor.matmul(
                            att, lhsT=vsb[:, jt, h], rhs=pt,
                            start=(jt == 0), stop=(jt == NT - 1))
                        nc.tensor.matmul(
                            den, lhsT=ones, rhs=pt,
                            start=(jt == 0), stop=(jt == NT - 1))
                    rec = rcp.tile([128, QB], f32, tag="r")
                    nc.vector.reciprocal(rec, den)
                    nc.vector.tensor_mul(attnT[:, h, qsl], att, rec)

        if phases == 2:
            nc.sync.dma_start(out=dbg_d[3], in_=attnT.rearrange("p h s -> p (h s)"))

        # ---------------- phase 3: output projection -------------------------
        with (
            tc.tile_pool(name="wop", bufs=1) as wop,
            tc.tile_pool(name="op", bufs=2) as op,
            tc.tile_pool(name="ops", bufs=3, space="PSUM") as ops,
        ):
            if phases >= 3:
                woi = wop.tile([128, HL, DM], f16, tag="woi")
                nc.sync.dma_start(out=woi, in_=woi_d[:])
                wot = wop.tile([128, HL, DM], f16, tag="wot")
                nc.sync.dma_start(out=wot, in_=wot_d[:])
            for t in range(NT if phases >= 3 else 0):
                wo = wot if t < NTT else woi
                ot = op.tile([128, DM], f32, tag="o")
                for oc in range(NOC):
                    osl = slice(oc * OC, (oc + 1) * OC)
                    po = ops.tile([128, OC], f32, tag="po")
                    for h in range(HL):
                        nc.tensor.matmul(
                            po, lhsT=attnT[:, h, t * 128:(t + 1) * 128],
                            rhs=wo[:, h, osl], start=(h == 0), stop=(h == HL - 1))
                    nc.scalar.copy(out=ot[:, osl], in_=po)
                nc.sync.dma_start(out=out_d[t * 128:(t + 1) * 128, :], in_=ot)

    nc.compile()
    return nc


def _prep_rope(rope_f, g):
    f = np.asarray(rope_f, dtype=np.float32)[:, 0]   # [s, 64, 2, 2]
    ge, go = g[0::2], g[1::2]
    return np.stack(
        [f[:, :, 0, 0] * ge, f[:, :, 0, 1] * go,
         f[:, :, 1, 0] * ge, f[:, :, 1, 1] * go], axis=1)  # [s, 4, 64]


def _host_prep(inputs):
    f16 = np.float16
    inp = {k: np.asarray(v, dtype=np.float32) for k, v in inputs.items()}
    Xj = np.concatenate([inp["encoder_hidden_states"][0], inp["hidden_states"][0]], 0)
    xh = np.ascontiguousarray(
        Xj.T.reshape(NCH, 128, NT, 128).transpose(2, 1, 0, 3).astype(f16))

    ropeq = np.ascontiguousarray(np.concatenate(
        [_prep_rope(inp["txt_rope"], inp["gaq"]),
         _prep_rope(inp["img_rope"], inp["gq"])], 0
    ).reshape(NT, 128, 4, 64).astype(f16))
    ropek = np.ascontiguousarray(np.concatenate(
        [_prep_rope(inp["txt_rope"], inp["gak"]),
         _prep_rope(inp["img_rope"], inp["gk"])], 0
    ).reshape(NT, 128, 4, 64).astype(f16))

    def wcat(wq, wk, wv, hs):
        wq = wq[:, hs].reshape(DM, HL, DH)[:, :, PERM].reshape(DM, HL * DH)
        wk = wk[:, hs].reshape(DM, HL, DH)[:, :, PERM].reshape(DM, HL * DH)
        wv = wv[:, hs]
        cat = np.concatenate([wq, wk, wv], axis=1)          # [DM, 3*HL*DH]
        return np.ascontiguousarray(
            cat.reshape(NCH, 128, 3 * HL * DH).transpose(1, 0, 2).astype(f16))

    in_maps = []
    for c in range(NCORES):
        hs = slice(c * HL * DH, (c + 1) * HL * DH)
        wimg = wcat(inp["Wq"], inp["Wk"], inp["Wv"], hs)
        wtxt = wcat(inp["Waq"], inp["Wak"], inp["Wav"], hs)
        woi = np.ascontiguousarray(
            inp["Wo"][hs].reshape(HL, DH, DM).transpose(1, 0, 2).astype(f16))
        wot = np.ascontiguousarray(
            inp["Wao"][hs].reshape(HL, DH, DM).transpose(1, 0, 2).astype(f16))
        in_maps.append({
            "x": xh, "wimg": wimg, "wtxt": wtxt,
            "ropeq": ropeq, "ropek": ropek, "woi": woi, "wot": wot,
        })
    return in_maps, inp


def kernel(**inputs):
    if "nc" not in _cached:
        _cached["nc"] = _build_nc()
    nc = _cached["nc"]
    in_maps, inp = _host_prep(inputs)
    res = run_bass_kernel_spmd(nc, in_maps, list(range(NCORES)))
    total = np.zeros((S, DM), np.float32)
    for r in res.results:
        total += r["out"]
    txt = total[:S_TXT] + inp["bao"]
    img = total[S_TXT:] + inp["bo"]
    return (img[None].astype(np.float32), txt[None].astype(np.float32))
